# revision 8
# baseline (speedup 1.0000x reference)
"""GCN (3-layer graph conv) on 8 Trainium2 NeuronCores.

v2 strategy (dst-sharded, big-call gathers, fp8 DoubleRow aggregation):
- Nodes sharded across 8 cores (12500 each, padded 12544 = 98 blocks of 128).
- Gather tables in fp8 (xt = ns*X, h1) / bf16-128 (y2); 4 windows of <=28672
  rows (int16 idx); ONE dma_gather per (4-block group x window) amortizes the
  ~1us SWDGE fixed overhead (100 calls/layer vs 686).
- Padding slots carry idx-0 descriptors + code 255 (one-hot zero row), so no
  memset and no stale-SBUF NaN risk; every tile is fully written.
- Segment-sum via fp8 DoubleRow matmul: lhsT = one-hot [128,2,128] f8, rhs =
  gathered [128,2,512] f8 (gather's plane-major layout matches DoubleRow),
  0.5 cyc/row. Per-group PSUM accumulation across windows (4 acc banks).
- Layer 2 transform-first: y2 = (ns*relu(h2)) @ W2 computed in layer 1's
  epilogue (bf16, 128-col padded); layer 2 aggregates 256B rows and applies
  nd + b2. Removes the f8 h2 table entirely (accuracy + traffic).
- W_lin folded into layer 0 (A(ns*X) @ (W_lin@W0)).
- Inter-layer exchange: chunked AllGathers (7 x 14336 rows) into one shared
  full-table tensor so gather windows can span 2 chunks.
"""

import math
import os
import numpy as np
import ml_dtypes

import concourse.bass as bass
import concourse.tile as tile
from concourse import bacc, mybir
from concourse.bass_utils import run_bass_kernel_spmd

USE_DR = os.environ.get("GCN_DR", "1") == "1"
BF16 = ml_dtypes.bfloat16
F8 = ml_dtypes.float8_e4m3fn


class Cfg:
    def __init__(self, N, E, D, C):
        self.N, self.E, self.D, self.C = N, E, D, C
        self.NC = 8
        self.SH = N // self.NC                       # 12500 nodes per core
        self.BLK = math.ceil(self.SH / 128)          # 98 blocks
        self.CHB = 14                                # blocks per collective chunk
        self.NCH = self.BLK // self.CHB              # 7 chunks
        self.PS = self.BLK * 128                     # 12544 padded shard rows
        self.CHR = self.CHB * 128                    # 1792 rows per chunk
        self.RT = self.NC * self.PS                  # 100352 padded table rows
        self.CREG = self.NC * self.CHR               # 14336 rows per chunk region
        # gather windows: pairs of chunk regions (<=32768 rows for int16 idx)
        self.WINB = [i * self.CREG for i in range(self.NCH + 1)]
        self.NW = self.NCH
        self.S = 4                                   # blocks per PSUM group
        self.NG = math.ceil(self.BLK / self.S)       # 25 groups
        self.NO2 = 128                               # padded y2 cols (bf16, 256B)


CFG = Cfg(N=100000, E=3200000, D=512, C=40)


def _rows_of(v, cfg):
    """Padded table row of node v under the [chunk][core][row] layout."""
    c = v // cfg.SH
    l = v % cfg.SH
    k = l // cfg.CHR
    r = l % cfg.CHR
    return (k * cfg.CREG) + c * cfg.CHR + r


def _preprocess(cfg, features, src, dst, W_lin, b_lin, W0, b0, W1, b1, W2, b2):
    N, E, D, C = cfg.N, cfg.E, cfg.D, cfg.C
    NW, S = cfg.NW, cfg.S
    deg_out = np.bincount(src, minlength=N).astype(np.float32)
    deg_in = np.bincount(dst, minlength=N).astype(np.float32)
    ns = np.maximum(deg_out, 1.0) ** -0.5
    nd = np.maximum(deg_in, 1.0) ** -0.5

    rows = _rows_of(np.arange(N, dtype=np.int64), cfg)
    xt = np.zeros((cfg.RT, D), dtype=F8)
    xt[rows] = (features * ns[:, None]).astype(F8)

    wc = (W_lin @ W0).astype(np.float32)

    def wtile(W, no):
        return np.ascontiguousarray(
            np.asarray(W, np.float32).reshape(D // 128, 128, no).transpose(1, 0, 2)
        ).astype(BF16)

    W2p = np.zeros((D, cfg.NO2), np.float32)
    W2p[:, :C] = W2

    srow = _rows_of(src.astype(np.int64), cfg)
    winb = np.asarray(cfg.WINB, np.int64)
    swin = np.searchsorted(winb, srow, side="right") - 1
    srel = srow - winb[swin]
    assert srel.max() < 32768

    # per-core edge grouping by (dst block, src window)
    per_core = []
    counts_all = np.zeros((cfg.NC, cfg.BLK * NW), dtype=np.int64)
    for c in range(cfg.NC):
        sel = (dst >= c * cfg.SH) & (dst < (c + 1) * cfg.SH)
        l = dst[sel].astype(np.int64) - c * cfg.SH
        b = l // 128
        code = (l % 128).astype(np.float32)
        key = b * NW + swin[sel]
        order = np.argsort(key, kind="stable")
        counts_all[c] = np.bincount(key[order], minlength=cfg.BLK * NW)
        per_core.append((key[order], srel[sel][order], code[order]))

    # uniform (SPMD) tile counts, rounded to even for DoubleRow pairing
    Vmax = counts_all.max(axis=0)
    T_bw = np.ceil(Vmax / 128.0).astype(np.int64)
    T_bw += T_bw % 2
    T_bw = T_bw.reshape(cfg.BLK, NW)
    assert (T_bw.sum(axis=1) >= 2).all()

    # global tile order: group -> window -> block
    groups = [list(range(S * i, min(S * i + S, cfg.BLK))) for i in range(cfg.NG)]
    tile_col = np.zeros((cfg.BLK, NW), np.int64)
    gw_tile0 = np.zeros((cfg.NG, NW), np.int64)   # first tile col of (g,w)
    gw_tiles = np.zeros((cfg.NG, NW), np.int64)   # tiles in (g,w)
    t = 0
    for gi, g in enumerate(groups):
        for w in range(NW):
            gw_tile0[gi, w] = t
            for b in g:
                tile_col[b, w] = t
                t += T_bw[b, w]
            gw_tiles[gi, w] = t - gw_tile0[gi, w]
    Ltiles = t
    eoff = (tile_col * 128).reshape(-1)           # idx elem offset per (b,w) key

    idx_arrs, code_arrs = [], []
    codes_flat_shape = Ltiles * 128
    for c in range(cfg.NC):
        key_s, rel_s, code_s = per_core[c]
        cnt_off = np.concatenate([[0], np.cumsum(counts_all[c])])
        pos_in_grp = np.arange(len(key_s), dtype=np.int64) - cnt_off[key_s]
        destp = eoff[key_s] + pos_in_grp
        idx_flat = np.zeros(codes_flat_shape, np.int16)     # filler: row 0 of window
        code_flat = np.full(codes_flat_shape, 255.0, np.float32)
        idx_flat[destp] = rel_s.astype(np.int16)
        code_flat[destp] = code_s
        idx16 = idx_flat.reshape(-1, 16).T                   # [16, L/16]
        idx_arrs.append(np.ascontiguousarray(np.tile(idx16, (8, 1))))
        code_arrs.append(
            np.ascontiguousarray(code_flat.reshape(-1, 128).T).astype(BF16)
        )

    nd_t, ns_t = [], []
    for c in range(cfg.NC):
        pad = np.zeros(cfg.PS, np.float32)
        pad[: cfg.SH] = nd[c * cfg.SH : (c + 1) * cfg.SH]
        nd_t.append(np.ascontiguousarray(pad.reshape(cfg.BLK, 128).T))
        pad2 = np.zeros(cfg.PS, np.float32)
        pad2[: cfg.SH] = ns[c * cfg.SH : (c + 1) * cfg.SH]
        ns_t.append(np.ascontiguousarray(pad2.reshape(cfg.BLK, 128).T))

    iota = np.ascontiguousarray(
        np.broadcast_to(
            np.tile(np.arange(128, dtype=np.float32), 8)[None, :], (128, 1024)
        )
    ).astype(BF16)
    ident = np.eye(128, dtype=np.float32).astype(BF16)

    bias = {}
    bias["b0row"] = np.asarray(b0, np.float32)
    bias["b1row"] = np.asarray(b1, np.float32)
    bias["b2row"] = np.asarray(b2, np.float32)
    bias["bw0row"] = (np.asarray(b_lin, np.float32) @ W0).astype(np.float32)
    if np.any(bias["bw0row"]):
        cvec = np.bincount(dst, weights=ns[src], minlength=N).astype(np.float32) * nd
        cn_t = []
        for c in range(cfg.NC):
            pad = np.zeros(cfg.PS, np.float32)
            pad[: cfg.SH] = cvec[c * cfg.SH : (c + 1) * cfg.SH]
            cn_t.append(np.ascontiguousarray(pad.reshape(cfg.BLK, 128).T))
        bias["cn_t"] = cn_t

    return dict(
        xt=xt,
        wc=wtile(wc, D), w1=wtile(W1, D), w2=wtile(W2p, cfg.NO2),
        idx=idx_arrs, codes=code_arrs, nd=nd_t, ns=ns_t,
        iota=iota, ident=ident,
        T_bw=T_bw, groups=groups, tile_col=tile_col,
        gw_tile0=gw_tile0, gw_tiles=gw_tiles, Ltiles=Ltiles, bias=bias,
    )


def _build(cfg, pre, bias_en):
    D, C, NW = cfg.D, cfg.C, cfg.NW
    T_bw = pre["T_bw"]
    groups = pre["groups"]
    tile_col = pre["tile_col"]
    gw_tile0 = pre["gw_tile0"]
    gw_tiles = pre["gw_tiles"]
    Ltiles = int(pre["Ltiles"])
    max_gw = int(gw_tiles.max())

    nc = bacc.Bacc("TRN2", num_devices=cfg.NC, num_swdge_queues=4)
    f32, bf16, i16 = mybir.dt.float32, mybir.dt.bfloat16, mybir.dt.int16
    f8 = mybir.dt.float8e4
    AF = mybir.ActivationFunctionType

    xt = nc.declare_dram_parameter("xt", [cfg.RT, D], f8, isOutput=False)
    idxs = nc.declare_dram_parameter("idxs", [128, Ltiles * 8], i16, isOutput=False)
    codes = nc.declare_dram_parameter("codes", [128, Ltiles], bf16, isOutput=False)
    wc = nc.declare_dram_parameter("wc", [128, D // 128, D], bf16, isOutput=False)
    w1 = nc.declare_dram_parameter("w1", [128, D // 128, D], bf16, isOutput=False)
    w2 = nc.declare_dram_parameter("w2", [128, D // 128, cfg.NO2], bf16, isOutput=False)
    ndp = nc.declare_dram_parameter("nd", [128, cfg.BLK], f32, isOutput=False)
    nsp = nc.declare_dram_parameter("ns", [128, cfg.BLK], f32, isOutput=False)
    iota = nc.declare_dram_parameter("iota", [128, 8, 128], bf16, isOutput=False)
    ident = nc.declare_dram_parameter("ident", [128, 128], bf16, isOutput=False)
    bias_p = {}
    if bias_en["b0"]:
        bias_p["b0row"] = nc.declare_dram_parameter("b0row", [128, D], f32, isOutput=False)
    if bias_en["b1"]:
        bias_p["b1row"] = nc.declare_dram_parameter("b1row", [128, D], f32, isOutput=False)
    if bias_en["b2"]:
        bias_p["b2row"] = nc.declare_dram_parameter("b2row", [128, C], f32, isOutput=False)
    if bias_en["blin"]:
        bias_p["bw0row"] = nc.declare_dram_parameter("bw0row", [128, D], f32, isOutput=False)
        bias_p["cn"] = nc.declare_dram_parameter("cn", [128, cfg.BLK], f32, isOutput=False)
    out = nc.declare_dram_parameter("out", [cfg.PS, C], f32, isOutput=True)

    h1s = [nc.dram_tensor(f"h1s_{i}", [cfg.CHR, D], f8) for i in range(cfg.NCH)]
    y2s = [nc.dram_tensor(f"y2s_{i}", [cfg.CHR, cfg.NO2], bf16) for i in range(cfg.NCH)]
    h1f = [nc.dram_tensor(f"h1f_{i}", [cfg.CREG, D], f8, addr_space="Shared") for i in range(cfg.NCH)]
    y2f = [nc.dram_tensor(f"y2f_{i}", [cfg.CREG, cfg.NO2], bf16, addr_space="Shared") for i in range(cfg.NCH)]

    cores = list(range(cfg.NC))
    DR = mybir.MatmulPerfMode.DoubleRow

    # first/last nonzero window per block (for PSUM start/stop flags)
    first_w = [int(np.nonzero(T_bw[b])[0][0]) for b in range(cfg.BLK)]
    last_w = [int(np.nonzero(T_bw[b])[0][-1]) for b in range(cfg.BLK)]

    with tile.TileContext(nc) as tc:
        with tc.tile_pool(name="const", bufs=1) as cp, \
             tc.tile_pool(name="idxp", bufs=4) as kp, \
             tc.tile_pool(name="stag", bufs=3) as sp, \
             tc.tile_pool(name="work", bufs=3) as wp, \
             tc.tile_pool(name="ohp", bufs=4) as op_, \
             tc.tile_pool(name="psA", bufs=1, space="PSUM") as psA, \
             tc.tile_pool(name="psT", bufs=2, space="PSUM") as psT, \
             tc.tile_pool(name="psD", bufs=2, space="PSUM") as psD:

            iota_sb = cp.tile([128, 8, 128], bf16)
            nc.sync.dma_start(out=iota_sb[:], in_=iota[:])
            ident_sb = cp.tile([128, 128], bf16)
            nc.sync.dma_start(out=ident_sb[:], in_=ident[:])
            nd_sb = cp.tile([128, cfg.BLK], f32)
            nc.sync.dma_start(out=nd_sb[:], in_=ndp[:])
            ns_sb = cp.tile([128, cfg.BLK], f32)
            nc.sync.dma_start(out=ns_sb[:], in_=nsp[:])
            codes_sb = cp.tile([128, Ltiles], bf16)
            nc.sync.dma_start(out=codes_sb[:], in_=codes[:])
            w_sb = {}
            for name, par, no in (("wc", wc, D), ("w1", w1, D), ("w2", w2, cfg.NO2)):
                t = cp.tile([128, D // 128, no], bf16, tag=f"w_{name}")
                nc.sync.dma_start(out=t[:], in_=par[:])
                w_sb[name] = t
            bias_sb = {}
            for nm in ("b0row", "b1row", "b2row", "bw0row"):
                if nm in bias_p:
                    no = C if nm == "b2row" else D
                    t = cp.tile([128, no], f32, tag=f"bias_{nm}")
                    nc.sync.dma_start(out=t[:], in_=bias_p[nm][:])
                    bias_sb[nm] = t
            if "cn" in bias_p:
                t = cp.tile([128, cfg.BLK], f32)
                nc.sync.dma_start(out=t[:], in_=bias_p["cn"][:])
                bias_sb["cn"] = t

            qn = [0]
            layers = (
                (0, xt, D, f8, "wc", "b0row"),
                (1, h1f, D, f8, "w1", "b1row"),
                (2, y2f, cfg.NO2, bf16, None, "b2row"),
            )
            for li, table, EL, gdt, wname, brow in layers:
                ACCW = D if li < 2 else cfg.NO2
                for gi, g in enumerate(groups):
                    accs = {}
                    for j in range(len(g)):
                        accs[j] = psA.tile(
                            [128, ACCW], f32, space="PSUM", tag=f"acc{j}", name=f"acc{j}"
                        )
                    for w in range(NW):
                        tg = int(gw_tiles[gi, w])
                        if tg == 0:
                            continue
                        c0gw = int(gw_tile0[gi, w])
                        idx_t = kp.tile([128, max_gw * 8], i16, tag="idx")
                        nc.sync.dma_start(
                            out=idx_t[:, : tg * 8],
                            in_=idxs[:, c0gw * 8 : (c0gw + tg) * 8],
                        )
                        stag = sp.tile([128, max_gw, EL], gdt, tag="stag")
                        if isinstance(table, list):
                            src_ap = table[w][:]
                        else:
                            wlo, whi = cfg.WINB[w], cfg.WINB[w + 1]
                            src_ap = table[wlo:whi, :]
                        CAPT = int(os.environ.get("GCN_CAP", "8"))
                        for ts in range(0, tg, CAPT):
                            tn = min(CAPT, tg - ts)
                            nc.gpsimd.dma_gather(
                                out_ap=stag[:, ts : ts + tn, :],
                                in_ap=src_ap,
                                idxs_ap=idx_t[:, ts * 8 : (ts + tn) * 8],
                                num_idxs=tn * 128,
                                num_idxs_reg=tn * 128,
                                elem_size=EL,
                                queue_num=qn[0] % 4,
                            )
                            qn[0] += 1
                        for j, b in enumerate(g):
                            tb = int(T_bw[b, w])
                            if tb == 0:
                                continue
                            t0 = int(tile_col[b, w]) - c0gw
                            cc = int(tile_col[b, w])
                            for o in range(0, tb, 8):
                                kk = min(8, tb - o)
                                oh = op_.tile([128, 8, 128], gdt, tag="oh")
                                nc.vector.tensor_tensor(
                                    out=oh[:, :kk, :],
                                    in0=codes_sb[:, cc + o : cc + o + kk].to_broadcast(
                                        [128, kk, 128]
                                    ),
                                    in1=iota_sb[:, :kk, :],
                                    op=mybir.AluOpType.is_equal,
                                )
                                for p in range(0, kk, 2):
                                    ti = o + p
                                    st = (w == first_w[b]) and (ti == 0)
                                    sp_ = (w == last_w[b]) and (ti == tb - 2)
                                    if li < 2 and USE_DR:
                                        nc.tensor.matmul(
                                            out=accs[j][:],
                                            lhsT=oh[:, p : p + 2, :],
                                            rhs=stag[:, t0 + ti : t0 + ti + 2, :],
                                            start=st, stop=sp_,
                                            perf_mode=DR,
                                        )
                                    else:
                                        nc.tensor.matmul(
                                            out=accs[j][:],
                                            lhsT=oh[:, p, :],
                                            rhs=stag[:, t0 + ti, :],
                                            start=st, stop=False,
                                        )
                                        nc.tensor.matmul(
                                            out=accs[j][:],
                                            lhsT=oh[:, p + 1, :],
                                            rhs=stag[:, t0 + ti + 1, :],
                                            start=False, stop=sp_,
                                        )
                    # dense epilogue per block of the group
                    for j, b in enumerate(g):
                        if li < 2:
                            wt = w_sb[wname]
                            mn = wp.tile([128, D], bf16, tag="mn")
                            nc.scalar.activation(
                                mn[:], accs[j][:], AF.Copy, scale=nd_sb[:, b : b + 1]
                            )
                            pT = psT.tile([128, D], bf16, space="PSUM", tag="pT")
                            for jj in range(D // 128):
                                nc.tensor.transpose(
                                    out=pT[:, jj * 128 : (jj + 1) * 128],
                                    in_=mn[:, jj * 128 : (jj + 1) * 128],
                                    identity=ident_sb[:],
                                )
                            lhsT = wp.tile([128, D], bf16, tag="lhsT")
                            nc.vector.tensor_copy(out=lhsT[:], in_=pT[:])
                            pd = psD.tile([128, D], f32, space="PSUM", tag="pd")
                            for jj in range(D // 128):
                                nc.tensor.matmul(
                                    out=pd[:],
                                    lhsT=lhsT[:, jj * 128 : (jj + 1) * 128],
                                    rhs=wt[:, jj, :],
                                    start=(jj == 0), stop=(jj == D // 128 - 1),
                                )
                            if li == 0 and "bw0row" in bias_sb:
                                tmp = wp.tile([128, D], f32, tag="btmp")
                                nc.vector.tensor_scalar_mul(
                                    tmp[:], bias_sb["bw0row"][:], bias_sb["cn"][:, b : b + 1]
                                )
                                nc.vector.tensor_add(pd[:], pd[:], tmp[:])
                            if brow in bias_sb:
                                nc.vector.tensor_add(pd[:], pd[:], bias_sb[brow][:])
                            ch, bl = b // cfg.CHB, b % cfg.CHB
                            if li == 0:
                                ht = wp.tile([128, D], f8, tag="ht")
                                nc.scalar.activation(
                                    ht[:], pd[:], AF.Relu, scale=ns_sb[:, b : b + 1]
                                )
                                nc.sync.dma_start(
                                    out=h1s[ch][bl * 128 : (bl + 1) * 128, :], in_=ht[:]
                                )
                            else:
                                htb = wp.tile([128, D], bf16, tag="htb")
                                nc.scalar.activation(
                                    htb[:], pd[:], AF.Relu, scale=ns_sb[:, b : b + 1]
                                )
                                pT2 = psT.tile([128, D], bf16, space="PSUM", tag="pT")
                                for jj in range(D // 128):
                                    nc.tensor.transpose(
                                        out=pT2[:, jj * 128 : (jj + 1) * 128],
                                        in_=htb[:, jj * 128 : (jj + 1) * 128],
                                        identity=ident_sb[:],
                                    )
                                lh2 = wp.tile([128, D], bf16, tag="lhsT")
                                nc.vector.tensor_copy(out=lh2[:], in_=pT2[:])
                                pd2 = psD.tile([128, cfg.NO2], f32, space="PSUM", tag="pd")
                                for jj in range(D // 128):
                                    nc.tensor.matmul(
                                        out=pd2[:],
                                        lhsT=lh2[:, jj * 128 : (jj + 1) * 128],
                                        rhs=w_sb["w2"][:, jj, :],
                                        start=(jj == 0), stop=(jj == D // 128 - 1),
                                    )
                                y2t = wp.tile([128, cfg.NO2], bf16, tag="y2t")
                                nc.scalar.activation(y2t[:], pd2[:], AF.Copy)
                                nc.sync.dma_start(
                                    out=y2s[ch][bl * 128 : (bl + 1) * 128, :], in_=y2t[:]
                                )
                            if bl == cfg.CHB - 1:
                                shard = h1s[ch] if li == 0 else y2s[ch]
                                full = h1f[ch] if li == 0 else y2f[ch]
                                nc.gpsimd.collective_compute(
                                    "AllGather",
                                    mybir.AluOpType.bypass,
                                    ins=[shard[:]],
                                    outs=[full[:]],
                                    replica_groups=[cores],
                                )
                        else:
                            ot = wp.tile([128, C], f32, tag="ot")
                            nc.scalar.activation(
                                ot[:], accs[j][:, :C], AF.Copy, scale=nd_sb[:, b : b + 1]
                            )
                            if brow in bias_sb:
                                nc.vector.tensor_add(ot[:], ot[:], bias_sb[brow][:])
                            nc.sync.dma_start(
                                out=out[b * 128 : (b + 1) * 128, :], in_=ot[:]
                            )
    nc.compile()
    return nc


_CACHE = {}


def _run(cfg, inputs, trace=False):
    pre = _preprocess(cfg, **inputs)
    bias_en = {
        "b0": bool(np.any(pre["bias"]["b0row"])),
        "b1": bool(np.any(pre["bias"]["b1row"])),
        "b2": bool(np.any(pre["bias"]["b2row"])),
        "blin": bool(np.any(pre["bias"]["bw0row"])),
    }
    key = (id(cfg), tuple(pre["T_bw"].reshape(-1)), tuple(sorted(bias_en.items())))
    if key not in _CACHE:
        _CACHE[key] = _build(cfg, pre, bias_en)
    nc = _CACHE[key]

    in_maps = []
    for c in range(cfg.NC):
        m = dict(
            xt=pre["xt"],
            idxs=pre["idx"][c], codes=pre["codes"][c],
            wc=pre["wc"], w1=pre["w1"], w2=pre["w2"],
            nd=pre["nd"][c], ns=pre["ns"][c],
            iota=pre["iota"], ident=pre["ident"],
        )
        rep = lambda v: np.ascontiguousarray(np.broadcast_to(v[None, :], (128, v.shape[0])))
        if bias_en["b0"]:
            m["b0row"] = rep(pre["bias"]["b0row"])
        if bias_en["b1"]:
            m["b1row"] = rep(pre["bias"]["b1row"])
        if bias_en["b2"]:
            m["b2row"] = rep(pre["bias"]["b2row"])
        if bias_en["blin"]:
            m["bw0row"] = rep(pre["bias"]["bw0row"])
            m["cn"] = pre["bias"]["cn_t"][c]
        in_maps.append(m)

    r = run_bass_kernel_spmd(nc, in_maps, list(range(cfg.NC)), trace=trace)
    outs = [np.asarray(r.results[c]["out"])[: cfg.SH] for c in range(cfg.NC)]
    full = np.concatenate(outs, axis=0)[: cfg.N]
    return full, r


def kernel(**inputs):
    inputs = {k: np.asarray(v) for k, v in inputs.items()}
    out, _ = _run(CFG, inputs)
    return out
